# revision 34
# baseline (speedup 1.0000x reference)
"""Trainium2 Bass kernel for nn_AudioNetwork_37512244363307.

Algorithm: the reference applies 4 sequential blocks of
  frame(hop 1024, win 2048) -> rfft -> per-(c,k) linear recurrence over
  frames -> irfft * hann -> overlap-add -> tanh(gain*x)
with identity channel mixing.  The per-channel transfer vectors are ~1%
sparse (<= 32 nonzero of 1025 coeffs), so each block reduces to:
  - forward: per hop-chunk j, a_j(k) = sum_n u_j[n] e^{-2pi i k n/2048}
    for the nonzero k only (matmul against a small DFT basis);
    S[i,k] = a_i(k) + (-1)^k a_{i+1}(k)
  - recurrence o[i] = (S[i] + o[i-1]) * t   (hardware tensor_tensor_scan)
  - synthesis: output chunk j = Ocat[j] @ G where Ocat stacks
    [Re o_j, Im o_j, Re o_{j-1}, Im o_{j-1}] and G folds the irfft basis,
    hann window and overlap-add of the two contributing frames.
Channels x batch are sharded over 8 NeuronCores (8 channels each); the
final sum over channels/blocks is accumulated on-core and reduced on host.
Matmuls run as float32r (full fp32 data, single-pass PE mode).
"""
import numpy as np

WS = 2048
STEP = 1024
NCOEF = WS // 2 + 1   # 1025
CPD = 64
NB = 4
B = 4
T = 131072
FRAMES = T // STEP    # 128
FR1 = FRAMES + 1      # 129: leading zero/reset column per batch
NK = 32               # padded nonzero-coeff slots per channel
NCORES = 8
CH_PER_CORE = CPD // NCORES  # 8
SUBS = STEP // 128    # 8
DVE_ACC = (0, 1, 2, 3, 4, 5, 6, 7)   # all acc adds on DVE (fp16 2x mode);
# GpSimd tensor ops take an exclusive lock on DVE's SBUF port, so routing
# adds there slows DVE down more than it helps.


def _hann():
    return 0.5 * (1.0 - np.cos(2.0 * np.pi * np.arange(WS) / WS))


def _make_tables(transfers):
    """Host-precomputed DFT/synthesis bases, per (block, channel).

    Returns arrays shaped for direct DMA into SBUF tiles:
      fwdb (NB, CPD, 128, SUBS, 2*NK)  lhsT for forward DFT
      synb (NB, CPD, 128, SUBS, 128)   lhsT for synthesis
      ttab (NB, CPD, 2*NK, B, FR1)     transfer broadcast, col 0 = 0 (reset)
      sgn  (NB, CPD, 2*NK, 1)          (-1)^k per slot
    """
    H = _hann()
    n1 = np.arange(STEP)
    fwdb = np.zeros((NB, CPD, 128, SUBS, 2 * NK), np.float32)
    synb = np.zeros((NB, CPD, 128, SUBS, 128), np.float32)
    ttab = np.zeros((NB, CPD, 2 * NK, B, FR1), np.float32)
    sgn = np.zeros((NB, CPD, 2 * NK, 1), np.float32)
    for i in range(NB):
        for c in range(CPD):
            t = transfers[i, c]
            ks = np.nonzero(t)[0]
            nk = len(ks)
            if nk > NK:
                raise ValueError("too many nonzero coeffs")
            kpad = np.zeros(NK, np.int64)
            kpad[:nk] = ks
            tpad = np.zeros(NK, np.float32)
            tpad[:nk] = t[ks]
            valid = np.zeros(NK, np.float32)
            valid[:nk] = 1.0
            th = 2.0 * np.pi * kpad[None, :] * n1[:, None] / WS  # (1024, NK)
            cos = np.cos(th) * valid
            sin = np.sin(th) * valid
            fwd = np.concatenate([cos, -sin], axis=1).astype(np.float32)
            fwdb[i, c] = fwd.reshape(SUBS, 128, 2 * NK).transpose(1, 0, 2)
            sign = np.where(kpad % 2 == 0, 1.0, -1.0).astype(np.float32)
            sgn[i, c, :NK, 0] = sign
            sgn[i, c, NK:, 0] = sign
            f = np.where(kpad == 0, 1.0 / WS, 2.0 / WS) * valid
            g1re = f[None, :] * H[:STEP, None] * np.cos(th)
            g1im = -f[None, :] * H[:STEP, None] * np.sin(th)
            g2re = f[None, :] * H[STEP:, None] * sign[None, :] * np.cos(th)
            g2im = -f[None, :] * H[STEP:, None] * sign[None, :] * np.sin(th)
            synth = np.concatenate(
                [g1re.T, g1im.T, g2re.T, g2im.T], axis=0).astype(np.float32)
            synb[i, c] = synth.reshape(128, SUBS, 128)
            t2 = np.concatenate([tpad, tpad])
            ttab[i, c, :, :, 1:] = np.broadcast_to(
                t2[:, None, None], (2 * NK, B, FRAMES))
    return fwdb, synb, ttab, sgn


def _build_bass(gains, skew=True):
    import concourse.bass as bass
    import concourse.mybir as mybir
    from concourse import bacc, tile

    f32 = mybir.dt.float32
    f16 = mybir.dt.float16
    # fp16 basis blob per (block, channel): fwd lhsT [128,8,64] then synth
    # lhsT [128,8,128]; the f32 ttab/sgn tables ride in a second small DMA.
    BLOBW = 1536
    nc = bacc.Bacc()
    xin = nc.declare_dram_parameter(
        "xin", [128, CH_PER_CORE, SUBS, B, FRAMES], f16, isOutput=False)
    blob = nc.declare_dram_parameter(
        "blob", [NB, CH_PER_CORE, 128, BLOBW], f16, isOutput=False)
    ttsg = nc.declare_dram_parameter(
        "ttsg", [NB, CH_PER_CORE // 2, 128, B * FR1 + 1], f32,
        isOutput=False)
    outa = nc.declare_dram_parameter(
        "outa", [NB, 128, SUBS, B, FRAMES], f16, isOutput=True)
    outb = nc.declare_dram_parameter(
        "outb", [NB, 128, SUBS, B, FRAMES], f16, isOutput=True)

    with tile.TileContext(nc) as tc:
        with (
            tc.tile_pool(name="res", bufs=CH_PER_CORE) as res_pool,
            tc.tile_pool(name="acc", bufs=2) as acc_pool,
            tc.tile_pool(name="basis", bufs=6) as basis_pool,
            tc.tile_pool(name="work", bufs=6) as work_pool,
            tc.tile_pool(name="fps", bufs=2, space=bass.MemorySpace.PSUM) as fps_pool,
            tc.tile_pool(name="sps", bufs=3, space=bass.MemorySpace.PSUM) as sps_pool,
        ):
            # sub-major layout: tanh writes and fwd matmul reads are
            # contiguous column ranges.
            res = [res_pool.tile([128, SUBS, B, FRAMES], f16, tag="res",
                                 name=f"res{c}")
                   for c in range(CH_PER_CORE)]
            warm = work_pool.tile([128, 16], f16, tag="warm", bufs=1)
            warmps = fps_pool.tile([16, 16], f32, tag="fps")
            nc.gpsimd.memset(warm[:], 0.0)
            for _ in range(50):
                nc.tensor.matmul(warmps[:], warm[:, 0:16], warm[:],
                                 start=True, stop=True)

            def front_half(i, p):
                """Paired front: channels (2p, 2p+1) share the fwd PSUM bank
                via column-group tiling, so S-build + scan run once per pair
                on all 128 partitions."""
                c0, c1 = 2 * p, 2 * p + 1
                if i == 0:
                    nc.sync.dma_start(res[c0][:], xin[:, c0])
                    nc.sync.dma_start(res[c1][:], xin[:, c1])
                bl0 = basis_pool.tile([128, BLOBW], f16, tag="bl0")
                bl1 = basis_pool.tile([128, BLOBW], f16, tag="bl1")
                nc.sync.dma_start(bl0[:], blob[i, c0])
                nc.sync.dma_start(bl1[:], blob[i, c1])
                tg = basis_pool.tile([128, B * FR1 + 1], f32, tag="tg")
                nc.sync.dma_start(tg[:], ttsg[i, p])
                fb0 = bl0[:, 0:512].rearrange('p (s m) -> p s m', s=SUBS)
                fb1 = bl1[:, 0:512].rearrange('p (s m) -> p s m', s=SUBS)
                sb0 = bl0[:, 512:1536].rearrange('p (s m) -> p s m', s=SUBS)
                sb1 = bl1[:, 512:1536].rearrange('p (s m) -> p s m', s=SUBS)
                tt = tg[:, 0:B * FR1].rearrange('p (b j) -> p b j', b=B)
                sg = tg[:, B * FR1:B * FR1 + 1]

                fwdps = fps_pool.tile([128, B, FRAMES], f32, tag="fps")
                for s in range(SUBS):
                    nc.tensor.matmul(
                        fwdps[0:64], fb0[:, s, :], res[c0][:, s, :, :],
                        start=(s == 0), stop=(s == SUBS - 1),
                        tile_position=(0, 0), skip_group_check=True)
                    nc.tensor.matmul(
                        fwdps[64:128], fb1[:, s, :], res[c1][:, s, :, :],
                        start=(s == 0), stop=(s == SUBS - 1),
                        tile_position=(0, 64), skip_group_check=True)
                # stile col (b, 1+i) = S[i] = a_i + sign * a_{i+1}; col (b,0)
                # is a reset column (t=0 there); memset keeps it finite.
                stile = work_pool.tile([128, B, FR1], f32, tag="stile")
                nc.gpsimd.memset(stile[:, :, 0:1], 0.0)
                nc.vector.tensor_copy(
                    stile[:, :, 1:FR1], fwdps[:, :, 0:FRAMES])
                nc.vector.scalar_tensor_tensor(
                    stile[:, :, 1:FRAMES], fwdps[:, :, 1:FRAMES], sg,
                    stile[:, :, 1:FRAMES],
                    mybir.AluOpType.mult, mybir.AluOpType.add)
                # one batched scan: both channels (partition halves) and all
                # b; col (b,0) has t=0 so state resets at batch boundaries.
                opair = work_pool.tile([128, B, FR1], f16, tag="opair")
                nc.vector.tensor_tensor_scan(
                    opair[:].rearrange('p b j -> p (b j)'),
                    stile[:].rearrange('p b j -> p (b j)'),
                    tt.rearrange('p b j -> p (b j)'),
                    0.0, mybir.AluOpType.add, mybir.AluOpType.mult)
                ocat0 = work_pool.tile([128, B, FR1], f16, tag="ocat0")
                ocat1 = work_pool.tile([128, B, FR1], f16, tag="ocat1")
                nc.sync.dma_start(
                    ocat0[0:64, :, 1:FR1], opair[0:64, :, 1:FR1])
                nc.sync.dma_start(
                    ocat0[64:128, :, 1:FR1], opair[0:64, :, 0:FRAMES])
                nc.sync.dma_start(
                    ocat1[0:64, :, 1:FR1], opair[64:128, :, 1:FR1])
                nc.sync.dma_start(
                    ocat1[64:128, :, 1:FR1], opair[64:128, :, 0:FRAMES])
                return (sb0, ocat0), (sb1, ocat1)

            def back_half(i, c, sb, ocat, acca, accb):
                synrhs = ocat[:, :, 1:FR1]
                for sp in range(SUBS // 2):
                    synps = sps_pool.tile([128, 2, B, FRAMES], f32, tag="sps")
                    for h in range(2):
                        nc.tensor.matmul(
                            synps[:, h], sb[:, 2 * sp + h, :], synrhs,
                            start=True, stop=True)
                    nc.scalar.activation(
                        res[c][:, 2 * sp:2 * sp + 2, :, :], synps[:],
                        mybir.ActivationFunctionType.Tanh,
                        scale=float(gains[i]))
                # two DVE-accumulated halves (shorter chains, host sums)
                acc = acca if c < 4 else accb
                if c % 4 == 0:
                    nc.sync.dma_start(acc[:], res[c][:])
                else:
                    nc.vector.tensor_add(acc[:], acc[:], res[c][:])

            from collections import deque
            pend_q = deque()
            for i in range(NB):
                acca = acc_pool.tile([128, SUBS, B, FRAMES], f16, tag="acca")
                accb = acc_pool.tile([128, SUBS, B, FRAMES], f16, tag="accb")
                for p in range(CH_PER_CORE // 2):
                    st0, st1 = front_half(i, p)
                    pend_q.append((i, 2 * p, st0[0], st0[1], acca, accb))
                    pend_q.append((i, 2 * p + 1, st1[0], st1[1], acca, accb))
                    while len(pend_q) > (4 if skew else 0):
                        pi, pc, psb, pocat, pacca, paccb = pend_q.popleft()
                        back_half(pi, pc, psb, pocat, pacca, paccb)
                        if pc == CH_PER_CORE - 1:
                            nc.sync.dma_start(outa[pi], pacca[:])
                            nc.sync.dma_start(outb[pi], paccb[:])
            while pend_q:
                pi, pc, psb, pocat, pacca, paccb = pend_q.popleft()
                back_half(pi, pc, psb, pocat, pacca, paccb)
                if pc == CH_PER_CORE - 1:
                    nc.sync.dma_start(outa[pi], pacca[:])
                    nc.sync.dma_start(outb[pi], paccb[:])
    nc.compile()
    return nc


def _prep_inputs(x, transfers):
    fwdb, synb, ttab, sgn = _make_tables(transfers)
    blob = np.zeros((NB, CPD, 128, 1536), np.float16)
    blob[:, :, :, 0:512] = fwdb.reshape(NB, CPD, 128, 512).astype(np.float16)
    blob[:, :, :, 512:1536] = synb.reshape(
        NB, CPD, 128, 1024).astype(np.float16)
    ttsg = np.concatenate(
        [ttab.reshape(NB, CPD, 64, 516), sgn], axis=3).astype(np.float32)
    ttsg = ttsg.reshape(NB, CPD // 2, 128, 517)
    # x (B, CPD, T) -> [n', c, b, s, j] with t = j*1024 + s*128 + n'
    x5 = x.reshape(B, CPD, FRAMES, SUBS, 128)
    xt = np.ascontiguousarray(
        np.transpose(x5, (4, 1, 3, 0, 2)).astype(np.float16))
    in_maps = []
    for core in range(NCORES):
        cl = core * CH_PER_CORE
        ch = cl + CH_PER_CORE
        in_maps.append({
            "xin": np.ascontiguousarray(xt[:, cl:ch]),
            "blob": np.ascontiguousarray(blob[:, cl:ch]),
            "ttsg": np.ascontiguousarray(ttsg[:, cl // 2:ch // 2]),
        })
    return in_maps


def _combine(x, outs, mixer):
    # outs: per-core list of (NB, 128, B, SUBS, FRAMES) block partials
    mv = np.exp(mixer - np.max(mixer))
    mv = (mv / mv.sum()).astype(np.float32)
    total = np.zeros((NB, 128, SUBS, B, FRAMES), np.float32)
    for o in outs:
        total += np.asarray(o, np.float32)
    mixed = np.einsum('l...,l->...', total, mv[1:])  # (128, SUBS, B, FRAMES)
    y = np.transpose(mixed, (2, 3, 1, 0)).reshape(B, T)  # b, j, s, n'
    y = y + mv[0] * x.sum(axis=1)
    return np.ascontiguousarray(y[:, None, :]).astype(np.float32)


def _kernel_np_fallback(x, transfers, mixer_matrices, gains, mixer):
    H = _hann()
    frames = x.shape[-1] // STEP
    mv = np.exp(mixer - np.max(mixer))
    mv = mv / mv.sum()
    outputs = [x.astype(np.float32)]
    inp = x.astype(np.float32)
    idx = np.arange(frames)[:, None] * STEP + np.arange(WS)[None, :]
    for i in range(NB):
        xm = np.einsum('bct,cd->bdt', inp, mixer_matrices[i])
        xp = np.pad(xm, ((0, 0), (0, 0), (0, WS - STEP)))
        windowed = xp[..., idx]
        spec = np.fft.rfft(windowed, axis=-1)
        Tc = transfers[i].astype(spec.dtype)
        o = np.zeros(spec.shape[:2] + (spec.shape[3],), spec.dtype)
        outspec = np.empty_like(spec)
        for fidx in range(frames):
            o = (spec[:, :, fidx] + o) * Tc[None]
            outspec[:, :, fidx] = o
        wins = np.fft.irfft(outspec, n=WS, axis=-1) * H
        L = (frames - 1) * STEP + WS
        samples = np.zeros(xm.shape[:2] + (L,), np.float32)
        for fidx in range(frames):
            samples[..., fidx * STEP:fidx * STEP + WS] += \
                wins[:, :, fidx].astype(np.float32)
        inp = np.tanh(samples[..., :x.shape[-1]] * gains[i]).astype(np.float32)
        outputs.append(inp)
    result = np.stack(outputs, axis=-1)
    mixed = (result * mv[None, None, None, :]).sum(-1)
    return mixed.sum(axis=1, keepdims=True).astype(np.float32)


def _conforms(x, transfers, mixer_matrices, gains, mixer):
    try:
        if x.shape != (B, CPD, T) or transfers.shape != (NB, CPD, NCOEF):
            return False
        if mixer_matrices.shape != (NB, CPD, CPD) or gains.shape != (NB,):
            return False
        eye = np.eye(CPD, dtype=np.float32)
        if not all(np.array_equal(mixer_matrices[i], eye) for i in range(NB)):
            return False
        if (transfers != 0).sum(axis=-1).max() > NK:
            return False
        # k = WS/2 (Nyquist) term would need a different irfft scale
        if np.any(transfers[:, :, NCOEF - 1] != 0):
            return False
        return True
    except Exception:
        return False


_CACHE = {}


def kernel(**inputs):
    x = np.asarray(inputs["x"], np.float32)
    transfers = np.asarray(inputs["transfers"], np.float32)
    mixer_matrices = np.asarray(inputs["mixer_matrices"], np.float32)
    gains = np.asarray(inputs["gains"], np.float32)
    mixer = np.asarray(inputs["mixer"], np.float32)
    if not _conforms(x, transfers, mixer_matrices, gains, mixer):
        return _kernel_np_fallback(x, transfers, mixer_matrices, gains, mixer)

    from concourse.bass_utils import run_bass_kernel_spmd
    in_maps = _prep_inputs(x, transfers)
    key = gains.tobytes()
    if key not in _CACHE:
        _CACHE[key] = _build_bass(gains)
    nc = _CACHE[key]
    res = run_bass_kernel_spmd(nc, in_maps, list(range(NCORES)))
    outs = [res.results[i]["outa"] for i in range(NCORES)]
    outs += [res.results[i]["outb"] for i in range(NCORES)]
    return _combine(x, outs, mixer)


# revision 35
# speedup vs baseline: 1.0088x; 1.0088x over previous
"""Trainium2 Bass kernel for nn_AudioNetwork_37512244363307.

Algorithm: the reference applies 4 sequential blocks of
  frame(hop 1024, win 2048) -> rfft -> per-(c,k) linear recurrence over
  frames -> irfft * hann -> overlap-add -> tanh(gain*x)
with identity channel mixing.  The per-channel transfer vectors are ~1%
sparse (<= 32 nonzero of 1025 coeffs), so each block reduces to:
  - forward: per hop-chunk j, a_j(k) = sum_n u_j[n] e^{-2pi i k n/2048}
    for the nonzero k only (matmul against a small DFT basis);
    S[i,k] = a_i(k) + (-1)^k a_{i+1}(k)
  - recurrence o[i] = (S[i] + o[i-1]) * t   (hardware tensor_tensor_scan)
  - synthesis: output chunk j = Ocat[j] @ G where Ocat stacks
    [Re o_j, Im o_j, Re o_{j-1}, Im o_{j-1}] and G folds the irfft basis,
    hann window and overlap-add of the two contributing frames.
Channels x batch are sharded over 8 NeuronCores (8 channels each); the
final sum over channels/blocks is accumulated on-core and reduced on host.
Matmuls run as float32r (full fp32 data, single-pass PE mode).
"""
import numpy as np

WS = 2048
STEP = 1024
NCOEF = WS // 2 + 1   # 1025
CPD = 64
NB = 4
B = 4
T = 131072
FRAMES = T // STEP    # 128
FR1 = FRAMES + 1      # 129: leading zero/reset column per batch
NK = 32               # padded nonzero-coeff slots per channel
NCORES = 8
CH_PER_CORE = CPD // NCORES  # 8
SUBS = STEP // 128    # 8
DVE_ACC = (0, 1, 2, 3, 4, 5, 6, 7)   # all acc adds on DVE (fp16 2x mode);
# GpSimd tensor ops take an exclusive lock on DVE's SBUF port, so routing
# adds there slows DVE down more than it helps.


def _hann():
    return 0.5 * (1.0 - np.cos(2.0 * np.pi * np.arange(WS) / WS))


def _make_tables(transfers):
    """Host-precomputed DFT/synthesis bases, per (block, channel).

    Returns arrays shaped for direct DMA into SBUF tiles:
      fwdb (NB, CPD, 128, SUBS, 2*NK)  lhsT for forward DFT
      synb (NB, CPD, 128, SUBS, 128)   lhsT for synthesis
      ttab (NB, CPD, 2*NK, B, FR1)     transfer broadcast, col 0 = 0 (reset)
      sgn  (NB, CPD, 2*NK, 1)          (-1)^k per slot
    """
    H = _hann()
    n1 = np.arange(STEP)
    fwdb = np.zeros((NB, CPD, 128, SUBS, 2 * NK), np.float32)
    synb = np.zeros((NB, CPD, 128, SUBS, 128), np.float32)
    ttab = np.zeros((NB, CPD, 2 * NK, B, FR1), np.float32)
    sgn = np.zeros((NB, CPD, 2 * NK, 1), np.float32)
    for i in range(NB):
        for c in range(CPD):
            t = transfers[i, c]
            ks = np.nonzero(t)[0]
            nk = len(ks)
            if nk > NK:
                raise ValueError("too many nonzero coeffs")
            kpad = np.zeros(NK, np.int64)
            kpad[:nk] = ks
            tpad = np.zeros(NK, np.float32)
            tpad[:nk] = t[ks]
            valid = np.zeros(NK, np.float32)
            valid[:nk] = 1.0
            th = 2.0 * np.pi * kpad[None, :] * n1[:, None] / WS  # (1024, NK)
            cos = np.cos(th) * valid
            sin = np.sin(th) * valid
            fwd = np.concatenate([cos, -sin], axis=1).astype(np.float32)
            fwdb[i, c] = fwd.reshape(SUBS, 128, 2 * NK).transpose(1, 0, 2)
            sign = np.where(kpad % 2 == 0, 1.0, -1.0).astype(np.float32)
            sgn[i, c, :NK, 0] = sign
            sgn[i, c, NK:, 0] = sign
            f = np.where(kpad == 0, 1.0 / WS, 2.0 / WS) * valid
            g1re = f[None, :] * H[:STEP, None] * np.cos(th)
            g1im = -f[None, :] * H[:STEP, None] * np.sin(th)
            g2re = f[None, :] * H[STEP:, None] * sign[None, :] * np.cos(th)
            g2im = -f[None, :] * H[STEP:, None] * sign[None, :] * np.sin(th)
            synth = np.concatenate(
                [g1re.T, g1im.T, g2re.T, g2im.T], axis=0).astype(np.float32)
            synb[i, c] = synth.reshape(128, SUBS, 128)
            t2 = np.concatenate([tpad, tpad])
            ttab[i, c, :, :, 1:] = np.broadcast_to(
                t2[:, None, None], (2 * NK, B, FRAMES))
    return fwdb, synb, ttab, sgn


def _build_bass(gains, skew=True):
    import concourse.bass as bass
    import concourse.mybir as mybir
    from concourse import bacc, tile

    f32 = mybir.dt.float32
    f16 = mybir.dt.float16
    # fp16 basis blob per (block, channel): fwd lhsT [128,8,64] then synth
    # lhsT [128,8,128]; the f32 ttab/sgn tables ride in a second small DMA.
    BLOBW = 1536
    nc = bacc.Bacc()
    xin = nc.declare_dram_parameter(
        "xin", [128, CH_PER_CORE, SUBS, B, FRAMES], f16, isOutput=False)
    blob = nc.declare_dram_parameter(
        "blob", [NB, CH_PER_CORE, 128, BLOBW], f16, isOutput=False)
    ttsg = nc.declare_dram_parameter(
        "ttsg", [NB, CH_PER_CORE // 2, 128, B * FR1 + 1], f32,
        isOutput=False)
    outa = nc.declare_dram_parameter(
        "outa", [NB, 128, SUBS, B, FRAMES], f16, isOutput=True)
    outb = nc.declare_dram_parameter(
        "outb", [NB, 128, SUBS, B, FRAMES], f16, isOutput=True)

    with tile.TileContext(nc) as tc:
        with (
            tc.tile_pool(name="res", bufs=CH_PER_CORE) as res_pool,
            tc.tile_pool(name="acc", bufs=2) as acc_pool,
            tc.tile_pool(name="basis", bufs=6) as basis_pool,
            tc.tile_pool(name="work", bufs=6) as work_pool,
            tc.tile_pool(name="fps", bufs=2, space=bass.MemorySpace.PSUM) as fps_pool,
            tc.tile_pool(name="sps", bufs=3, space=bass.MemorySpace.PSUM) as sps_pool,
        ):
            # sub-major layout: tanh writes and fwd matmul reads are
            # contiguous column ranges.
            res = [res_pool.tile([128, SUBS, B, FRAMES], f16, tag="res",
                                 name=f"res{c}")
                   for c in range(CH_PER_CORE)]
            warm = work_pool.tile([128, 16], f16, tag="warm", bufs=1)
            warmps = fps_pool.tile([16, 16], f32, tag="fps")
            nc.gpsimd.memset(warm[:], 0.0)
            for _ in range(50):
                nc.tensor.matmul(warmps[:], warm[:, 0:16], warm[:],
                                 start=True, stop=True)

            def front_half(i, p):
                """Paired front: channels (2p, 2p+1) share the fwd PSUM bank
                via column-group tiling, so S-build + scan run once per pair
                on all 128 partitions."""
                c0, c1 = 2 * p, 2 * p + 1
                if i == 0:
                    nc.sync.dma_start(res[c0][:], xin[:, c0])
                    nc.sync.dma_start(res[c1][:], xin[:, c1])
                bl0 = basis_pool.tile([128, BLOBW], f16, tag="bl0")
                bl1 = basis_pool.tile([128, BLOBW], f16, tag="bl1")
                nc.sync.dma_start(bl0[:], blob[i, c0])
                nc.sync.dma_start(bl1[:], blob[i, c1])
                tg = basis_pool.tile([128, B * FR1 + 1], f32, tag="tg")
                nc.sync.dma_start(tg[:], ttsg[i, p])
                fb0 = bl0[:, 0:512].rearrange('p (s m) -> p s m', s=SUBS)
                fb1 = bl1[:, 0:512].rearrange('p (s m) -> p s m', s=SUBS)
                sb0 = bl0[:, 512:1536].rearrange('p (s m) -> p s m', s=SUBS)
                sb1 = bl1[:, 512:1536].rearrange('p (s m) -> p s m', s=SUBS)
                tt = tg[:, 0:B * FR1].rearrange('p (b j) -> p b j', b=B)
                sg = tg[:, B * FR1:B * FR1 + 1]

                fwdps = fps_pool.tile([128, B, FRAMES], f32, tag="fps")
                for s in range(SUBS):
                    nc.tensor.matmul(
                        fwdps[0:64], fb0[:, s, :], res[c0][:, s, :, :],
                        start=(s == 0), stop=(s == SUBS - 1),
                        tile_position=(0, 0), skip_group_check=True)
                    nc.tensor.matmul(
                        fwdps[64:128], fb1[:, s, :], res[c1][:, s, :, :],
                        start=(s == 0), stop=(s == SUBS - 1),
                        tile_position=(0, 64), skip_group_check=True)
                # stile col (b, 1+i) = S[i] = a_i + sign * a_{i+1}; col (b,0)
                # is a reset column (t=0 there); memset keeps it finite.
                stile = work_pool.tile([128, B, FR1], f32, tag="stile")
                nc.gpsimd.memset(stile[:, :, 0:1], 0.0)
                nc.vector.tensor_copy(
                    stile[:, :, 1:FR1], fwdps[:, :, 0:FRAMES])
                nc.vector.scalar_tensor_tensor(
                    stile[:, :, 1:FRAMES], fwdps[:, :, 1:FRAMES], sg,
                    stile[:, :, 1:FRAMES],
                    mybir.AluOpType.mult, mybir.AluOpType.add)
                # one batched scan: both channels (partition halves) and all
                # b; col (b,0) has t=0 so state resets at batch boundaries.
                opair = work_pool.tile([128, B, FR1], f16, tag="opair")
                nc.vector.tensor_tensor_scan(
                    opair[:].rearrange('p b j -> p (b j)'),
                    stile[:].rearrange('p b j -> p (b j)'),
                    tt.rearrange('p b j -> p (b j)'),
                    0.0, mybir.AluOpType.add, mybir.AluOpType.mult)
                ocat0 = work_pool.tile([128, B, FR1], f16, tag="ocat0")
                ocat1 = work_pool.tile([128, B, FR1], f16, tag="ocat1")
                nc.sync.dma_start(
                    ocat0[0:64, :, 1:FR1], opair[0:64, :, 1:FR1])
                nc.sync.dma_start(
                    ocat0[64:128, :, 1:FR1], opair[0:64, :, 0:FRAMES])
                nc.sync.dma_start(
                    ocat1[0:64, :, 1:FR1], opair[64:128, :, 1:FR1])
                nc.sync.dma_start(
                    ocat1[64:128, :, 1:FR1], opair[64:128, :, 0:FRAMES])
                return (sb0, ocat0), (sb1, ocat1)

            def back_half(i, c, sb, ocat, acca, accb):
                synrhs = ocat[:, :, 1:FR1]
                for sp in range(SUBS // 2):
                    synps = sps_pool.tile([128, 2, B, FRAMES], f32, tag="sps")
                    for h in range(2):
                        nc.tensor.matmul(
                            synps[:, h], sb[:, 2 * sp + h, :], synrhs,
                            start=True, stop=True)
                    nc.scalar.activation(
                        res[c][:, 2 * sp:2 * sp + 2, :, :], synps[:],
                        mybir.ActivationFunctionType.Tanh,
                        scale=float(gains[i]))
                # two DVE-accumulated halves (shorter chains, host sums)
                acc = acca if c < 4 else accb
                if c % 4 == 0:
                    nc.sync.dma_start(acc[:], res[c][:])
                else:
                    nc.vector.tensor_add(acc[:], acc[:], res[c][:])

            from collections import deque
            pend_q = deque()
            for i in range(NB):
                acca = acc_pool.tile([128, SUBS, B, FRAMES], f16, tag="acca")
                accb = acc_pool.tile([128, SUBS, B, FRAMES], f16, tag="accb")
                for p in range(CH_PER_CORE // 2):
                    st0, st1 = front_half(i, p)
                    pend_q.append((i, 2 * p, st0[0], st0[1], acca, accb))
                    pend_q.append((i, 2 * p + 1, st1[0], st1[1], acca, accb))
                    while len(pend_q) > (2 if skew else 0):
                        pi, pc, psb, pocat, pacca, paccb = pend_q.popleft()
                        back_half(pi, pc, psb, pocat, pacca, paccb)
                        if pc == CH_PER_CORE - 1:
                            nc.sync.dma_start(outa[pi], pacca[:])
                            nc.sync.dma_start(outb[pi], paccb[:])
            while pend_q:
                pi, pc, psb, pocat, pacca, paccb = pend_q.popleft()
                back_half(pi, pc, psb, pocat, pacca, paccb)
                if pc == CH_PER_CORE - 1:
                    nc.sync.dma_start(outa[pi], pacca[:])
                    nc.sync.dma_start(outb[pi], paccb[:])
    nc.compile()
    return nc


def _prep_inputs(x, transfers):
    fwdb, synb, ttab, sgn = _make_tables(transfers)
    blob = np.zeros((NB, CPD, 128, 1536), np.float16)
    blob[:, :, :, 0:512] = fwdb.reshape(NB, CPD, 128, 512).astype(np.float16)
    blob[:, :, :, 512:1536] = synb.reshape(
        NB, CPD, 128, 1024).astype(np.float16)
    ttsg = np.concatenate(
        [ttab.reshape(NB, CPD, 64, 516), sgn], axis=3).astype(np.float32)
    ttsg = ttsg.reshape(NB, CPD // 2, 128, 517)
    # x (B, CPD, T) -> [n', c, b, s, j] with t = j*1024 + s*128 + n'
    x5 = x.reshape(B, CPD, FRAMES, SUBS, 128)
    xt = np.ascontiguousarray(
        np.transpose(x5, (4, 1, 3, 0, 2)).astype(np.float16))
    in_maps = []
    for core in range(NCORES):
        cl = core * CH_PER_CORE
        ch = cl + CH_PER_CORE
        in_maps.append({
            "xin": np.ascontiguousarray(xt[:, cl:ch]),
            "blob": np.ascontiguousarray(blob[:, cl:ch]),
            "ttsg": np.ascontiguousarray(ttsg[:, cl // 2:ch // 2]),
        })
    return in_maps


def _combine(x, outs, mixer):
    # outs: per-core list of (NB, 128, B, SUBS, FRAMES) block partials
    mv = np.exp(mixer - np.max(mixer))
    mv = (mv / mv.sum()).astype(np.float32)
    total = np.zeros((NB, 128, SUBS, B, FRAMES), np.float32)
    for o in outs:
        total += np.asarray(o, np.float32)
    mixed = np.einsum('l...,l->...', total, mv[1:])  # (128, SUBS, B, FRAMES)
    y = np.transpose(mixed, (2, 3, 1, 0)).reshape(B, T)  # b, j, s, n'
    y = y + mv[0] * x.sum(axis=1)
    return np.ascontiguousarray(y[:, None, :]).astype(np.float32)


def _kernel_np_fallback(x, transfers, mixer_matrices, gains, mixer):
    H = _hann()
    frames = x.shape[-1] // STEP
    mv = np.exp(mixer - np.max(mixer))
    mv = mv / mv.sum()
    outputs = [x.astype(np.float32)]
    inp = x.astype(np.float32)
    idx = np.arange(frames)[:, None] * STEP + np.arange(WS)[None, :]
    for i in range(NB):
        xm = np.einsum('bct,cd->bdt', inp, mixer_matrices[i])
        xp = np.pad(xm, ((0, 0), (0, 0), (0, WS - STEP)))
        windowed = xp[..., idx]
        spec = np.fft.rfft(windowed, axis=-1)
        Tc = transfers[i].astype(spec.dtype)
        o = np.zeros(spec.shape[:2] + (spec.shape[3],), spec.dtype)
        outspec = np.empty_like(spec)
        for fidx in range(frames):
            o = (spec[:, :, fidx] + o) * Tc[None]
            outspec[:, :, fidx] = o
        wins = np.fft.irfft(outspec, n=WS, axis=-1) * H
        L = (frames - 1) * STEP + WS
        samples = np.zeros(xm.shape[:2] + (L,), np.float32)
        for fidx in range(frames):
            samples[..., fidx * STEP:fidx * STEP + WS] += \
                wins[:, :, fidx].astype(np.float32)
        inp = np.tanh(samples[..., :x.shape[-1]] * gains[i]).astype(np.float32)
        outputs.append(inp)
    result = np.stack(outputs, axis=-1)
    mixed = (result * mv[None, None, None, :]).sum(-1)
    return mixed.sum(axis=1, keepdims=True).astype(np.float32)


def _conforms(x, transfers, mixer_matrices, gains, mixer):
    try:
        if x.shape != (B, CPD, T) or transfers.shape != (NB, CPD, NCOEF):
            return False
        if mixer_matrices.shape != (NB, CPD, CPD) or gains.shape != (NB,):
            return False
        eye = np.eye(CPD, dtype=np.float32)
        if not all(np.array_equal(mixer_matrices[i], eye) for i in range(NB)):
            return False
        if (transfers != 0).sum(axis=-1).max() > NK:
            return False
        # k = WS/2 (Nyquist) term would need a different irfft scale
        if np.any(transfers[:, :, NCOEF - 1] != 0):
            return False
        return True
    except Exception:
        return False


_CACHE = {}


def kernel(**inputs):
    x = np.asarray(inputs["x"], np.float32)
    transfers = np.asarray(inputs["transfers"], np.float32)
    mixer_matrices = np.asarray(inputs["mixer_matrices"], np.float32)
    gains = np.asarray(inputs["gains"], np.float32)
    mixer = np.asarray(inputs["mixer"], np.float32)
    if not _conforms(x, transfers, mixer_matrices, gains, mixer):
        return _kernel_np_fallback(x, transfers, mixer_matrices, gains, mixer)

    from concourse.bass_utils import run_bass_kernel_spmd
    in_maps = _prep_inputs(x, transfers)
    key = gains.tobytes()
    if key not in _CACHE:
        _CACHE[key] = _build_bass(gains)
    nc = _CACHE[key]
    res = run_bass_kernel_spmd(nc, in_maps, list(range(NCORES)))
    outs = [res.results[i]["outa"] for i in range(NCORES)]
    outs += [res.results[i]["outb"] for i in range(NCORES)]
    return _combine(x, outs, mixer)
